# revision 22
# baseline (speedup 1.0000x reference)
"""GNN message-passing (SAGEConv x3 + LayerNorm) Trainium2 kernel, 8 NeuronCores.

Strategy (graph/data parallel, per sharding hint):
  - Nodes sharded 8 ways by contiguous ranges (6250/core); edges owned by dst core.
  - Per layer: bf16 AllGather of node features, split into 3 row segments
    (blocks 0-23 / 24-46 / 47-48 of each core) emitted as soon as their blocks
    finish, so next-layer gathers for the bulk segments overlap the layer tail
    and the AllGather latency hides behind the tiny trailing segment ->
    per-core dma_gather of x[src] (edges sorted by (dst_block, segment, src))
    -> segment-sum via one-hot matmuls on the TensorEngine accumulating into
    PSUM per 128-dst block -> fused agg@Wl + x@Wr in transposed layout ->
    relu+bias on ScalarE -> residual+LN on DVE/ScalarE.
  - Gather indices are int16; each AllGather segment doubles as an index
    table (< 32768 rows each). Block edge lists are packed edge-granular
    with uniform-across-cores capacities; boundary tiles are shared between
    adjacent blocks via separate one-hot matrices. A host-side permutation
    shuffles nodes within each (core, segment) to equalize per-(block,segment)
    edge counts across cores (SPMD: one program for all 8 cores).
"""
import os
import sys

for _p in ('/opt/trn_rl_repo', '/root/.axon_site/_ro/trn_rl_repo'):
    if os.path.isdir(_p) and _p not in sys.path:
        sys.path.insert(0, _p)

import numpy as np

import concourse.bacc as bacc
import concourse.bass as bass
import concourse.tile as tile
from concourse import mybir
from concourse.bass_utils import run_bass_kernel_spmd
from concourse.masks import make_identity

N, E, D, L, POS_VOC = 50000, 800000, 128, 3, 1024
NCORES = 8
NLOC = N // NCORES            # 6250 nodes per core
NBLK = (NLOC + 127) // 128    # 49 dst blocks per core (last has 106)
LAST_VALID = NLOC - (NBLK - 1) * 128   # 106
SEG_B0 = [0, 24, 47]          # first block of each AllGather segment
SEG_NB = [24, 23, 2]          # blocks per segment
SEG_R0 = [0, 3072, 6016]      # first row of each segment within a core
SEG_NR = [3072, 2944, 234]    # rows per segment within a core
NSEG = 3
EPS = 1e-5
SQRT_D = float(np.sqrt(D))
CHUNK = 8                     # gather tiles per dma_gather call (1024 idxs max)
NQ = 4                        # SWDGE queues (parallel desc rings)
GBUFS = [int(x) for x in os.environ.get('GNN_GBUFS', '4,4,2').split(',')]

F32 = mybir.dt.float32
BF16 = mybir.dt.bfloat16
I16 = mybir.dt.int16
I32 = mybir.dt.int32
Alu = mybir.AluOpType
Act = mybir.ActivationFunctionType


def _wrap_idx(idx):
    """int16 gather index layout: logical j at [j%16, j//16], replicated to 128 partitions."""
    idx = np.asarray(idx, np.int16)
    n = len(idx)
    assert n % 16 == 0
    w = idx.reshape(-1, 16).T.copy()          # [16, n//16]
    return np.tile(w, (8, 1))                 # [128, n//16]


def _seg_of_rows(r):
    s = np.zeros_like(r)
    s[r >= SEG_R0[1]] = 1
    s[r >= SEG_R0[2]] = 2
    return s


def _balance_perm(edge):
    """Within each (core, segment), shuffle nodes among the segment's blocks so
    per-(core, block, src-segment) in-edge counts equalize across cores.
    Returns new_row[v] (global node id -> new global row)."""
    src = np.asarray(edge[0], np.int64)
    dst = np.asarray(edge[1], np.int64)
    s_core = src // NLOC
    s_r = src - s_core * NLOC
    cls = _seg_of_rows(s_r)
    deg = np.zeros((NSEG, N), np.int64)
    for h in range(NSEG):
        deg[h] = np.bincount(dst[cls == h], minlength=N)
    degt = deg.sum(axis=0)
    new_row = np.zeros(N, np.int64)
    for c in range(NCORES):
        base = c * NLOC
        for h in range(NSEG):
            nodes = np.arange(base + SEG_R0[h], base + SEG_R0[h] + SEG_NR[h])
            nodes = nodes[np.argsort(-degt[nodes], kind='stable')]
            nb = SEG_NB[h]
            capn = np.full(nb, 128, np.int64)
            if h == NSEG - 1:
                capn[nb - 1] = LAST_VALID
            load = np.zeros(nb, np.int64)
            cnt = np.zeros(nb, np.int64)
            slots = [[] for _ in range(nb)]
            for v in nodes:
                t = np.where(cnt < capn, load, 1 << 60)
                j = int(np.argmin(t))
                slots[j].append(v)
                load[j] += degt[v]
                cnt[j] += 1
            for j in range(nb):
                r0 = (SEG_B0[h] + j) * 128
                for k, v in enumerate(slots[j]):
                    new_row[v] = base + r0 + k
    return new_row


def _prepare(edge):
    """Host-side index preprocessing: per-core gather streams (one per segment
    table) with edge-granular per-block capacities uniform across cores, plus
    per-block one-hot column data (boundary tiles shared between blocks)."""
    src = np.asarray(edge[0], np.int64)
    dst = np.asarray(edge[1], np.int64)
    core = dst // NLOC
    dl = dst - core * NLOC
    blk = dl // 128
    col = dl - blk * 128
    s_core = src // NLOC
    s_r = src - s_core * NLOC
    cls = _seg_of_rows(s_r)
    seg_r0 = np.array(SEG_R0)[cls]
    seg_nr = np.array(SEG_NR)[cls]
    idxv = s_core * seg_nr + (s_r - seg_r0)

    # sort edges by (core, block, segment, src idx)
    key = (((core * NBLK + blk) * NSEG + cls) * (NCORES * max(SEG_NR) + 1)) + idxv
    order = np.argsort(key, kind='stable')
    g_idx, g_col = idxv[order], col[order]

    ngroups = NCORES * NBLK * NSEG
    gid = (core * NBLK + blk) * NSEG + cls
    counts = np.bincount(gid[order], minlength=ngroups).reshape(NCORES, NBLK, NSEG)
    caps = counts.max(axis=0)                              # [NBLK, NSEG]
    offs = np.zeros((NBLK + 1, NSEG), np.int64)
    offs[1:] = np.cumsum(caps, axis=0)
    T = [int((offs[NBLK][h] + 127) // 128) for h in range(NSEG)]

    spans = np.zeros((NBLK, NSEG), np.int64)
    col_offs = np.zeros((NBLK, NSEG), np.int64)
    tile0 = np.zeros((NBLK, NSEG), np.int64)
    acc = [0] * NSEG
    for b in range(NBLK):
        for h in range(NSEG):
            o0, o1 = int(offs[b][h]), int(offs[b + 1][h])
            t0 = o0 // 128
            t1 = (o1 - 1) // 128 + 1 if o1 > o0 else t0
            tile0[b][h] = t0
            spans[b][h] = t1 - t0
            col_offs[b][h] = acc[h]
            acc[h] += t1 - t0
    CS = [max(int(acc[h]), 1) for h in range(NSEG)]

    starts = np.concatenate([[0], np.cumsum(counts.reshape(-1))])
    per_core = []
    for c in range(NCORES):
        idx_s = [np.zeros(max(T[h], 1) * 128, np.int16) for h in range(NSEG)]
        col_s = [np.full(CS[h] * 128, -1.0, np.float32) for h in range(NSEG)]
        for b in range(NBLK):
            for h in range(NSEG):
                g = (c * NBLK + b) * NSEG + h
                s0, s1 = starts[g], starts[g + 1]
                n_e = s1 - s0
                if n_e == 0:
                    continue
                p0 = int(offs[b][h])
                idx_s[h][p0:p0 + n_e] = g_idx[s0:s1]
                q0 = int(col_offs[b][h]) * 128 + (p0 - int(tile0[b][h]) * 128)
                col_s[h][q0:q0 + n_e] = g_col[s0:s1]
        per_core.append(dict(
            idx=[_wrap_idx(idx_s[h]) for h in range(NSEG)],
            col=[col_s[h].reshape(-1, 128).T.copy() for h in range(NSEG)],
        ))
    return dict(T=T, CS=CS, spans=spans, tile0=tile0, col_offs=col_offs,
                per_core=per_core, span_max=max(1, int(spans.max())))


def _build(prep, ln_trivial):
    T, CS = prep['T'], prep['CS']
    spans, tile0, col_offs = prep['spans'], prep['tile0'], prep['col_offs']
    SPAN_MAX = prep['span_max']

    nc = bacc.Bacc('TRN2', num_devices=NCORES, num_swdge_queues=NQ,
                   dynamic_dma_scratch_size=int(os.environ.get('GNN_SCRATCH', '32768')))

    # ---- I/O ----
    node_emb_in = nc.dram_tensor("node_emb_in", [NLOC, D], F32, kind="ExternalInput")
    pos_idx_in = nc.dram_tensor("pos_idx_in", [128, 6272 // 16], I16, kind="ExternalInput")
    pos_table_in = nc.dram_tensor("pos_table_in", [POS_VOC, D], F32, kind="ExternalInput")
    idx_in = [nc.dram_tensor(f"idx{h}_in", [128, max(T[h], 1) * 8], I16,
                             kind="ExternalInput") for h in range(NSEG)]
    col_in = [nc.dram_tensor(f"col{h}_in", [128, CS[h]], F32, kind="ExternalInput")
              for h in range(NSEG)]
    wl_in = nc.dram_tensor("wl_in", [L, D, D], F32, kind="ExternalInput")
    wr_in = nc.dram_tensor("wr_in", [L, D, D], F32, kind="ExternalInput")
    blt_in = nc.dram_tensor("blt_in", [D, L], F32, kind="ExternalInput")
    ln_g_in = nc.dram_tensor("ln_g_in", [L + 1, D], F32, kind="ExternalInput")
    ln_b_in = nc.dram_tensor("ln_b_in", [L + 1, D], F32, kind="ExternalInput")
    y_out = nc.dram_tensor("y_out", [NLOC, D], F32, kind="ExternalOutput")

    F32R = mybir.dt.float32r
    x_my_bf = nc.dram_tensor("x_my_bf", [NLOC, D], F32R)
    # ping-pong gather tables by layer parity (avoids WAR with in-flight gathers)
    x_tbl = [[nc.dram_tensor(f"x_tbl{h}_{p}", [NCORES * SEG_NR[h], D],
                             F32R, addr_space="Shared")
              for p in range(2)] for h in range(NSEG)]

    with tile.TileContext(nc) as tc:
        with tc.tile_pool(name="const", bufs=1) as constp, \
             tc.tile_pool(name="xres", bufs=1) as xresp, \
             tc.tile_pool(name="work", bufs=3) as workp, \
             tc.tile_pool(name="psum", bufs=2, space="PSUM") as psump:

            # ---- constants ----
            idx_sb, col_sb = [], []
            for h in range(NSEG):
                isb = constp.tile([128, max(T[h], 1) * 8], I16, name=f"idx_sb{h}")
                nc.sync.dma_start(out=isb, in_=idx_in[h][:, :])
                idx_sb.append(isb)
                csb = constp.tile([128, CS[h]], F32, name=f"col_sb{h}")
                nc.sync.dma_start(out=csb, in_=col_in[h][:, :])
                col_sb.append(csb)
            pos_idx_sb = constp.tile([128, 6272 // 16], I16)
            nc.sync.dma_start(out=pos_idx_sb, in_=pos_idx_in[:, :])

            wl_sb = constp.tile([128, L, D], F32)
            nc.sync.dma_start(out=wl_sb, in_=wl_in[:, :, :].rearrange("l c f -> c l f"))
            wr_sb = constp.tile([128, L, D], F32)
            nc.sync.dma_start(out=wr_sb, in_=wr_in[:, :, :].rearrange("l c f -> c l f"))
            blt_sb = constp.tile([128, L], F32)
            nc.sync.dma_start(out=blt_sb, in_=blt_in[:, :])

            def bcast128(dram_row):   # replicate a [D] DRAM row across 128 partitions
                return bass.AP(tensor=dram_row.tensor, offset=dram_row.offset,
                               ap=[[0, 128]] + dram_row.ap)

            ln_g_sb = constp.tile([128, L + 1, D], F32)
            ln_b_sb = constp.tile([128, L + 1, D], F32)
            for i in range(L + 1):
                nc.sync.dma_start(out=ln_g_sb[:, i, :], in_=bcast128(ln_g_in[i, :]))
                nc.sync.dma_start(out=ln_b_sb[:, i, :], in_=bcast128(ln_b_in[i, :]))

            eps_sb = constp.tile([128, 1], F32)
            nc.vector.memset(eps_sb, EPS)
            ident = constp.tile([128, 128], F32)
            make_identity(nc, ident)

            iota_i = constp.tile([128, SPAN_MAX * 128], I32)
            nc.gpsimd.iota(iota_i, pattern=[[0, SPAN_MAX], [1, 128]], base=0,
                           channel_multiplier=0)
            iota_w = constp.tile([128, SPAN_MAX * 128], F32)
            nc.vector.tensor_copy(out=iota_w, in_=iota_i)

            # persistent x tiles (f32), one per block for fine-grained deps
            x_tiles = [xresp.tile([128, 128], F32, tag=f"x{t}", name=f"xres{t}")
                       for t in range(NBLK)]

            qn = [0]
            def next_q():
                q = qn[0] % NQ
                qn[0] += 1
                return q

            def layer_norm(src_ap, il, out_ap):
                stats = workp.tile([128, 6], F32, tag="stats")
                nc.vector.bn_stats(out=stats, in_=src_ap)
                mv = workp.tile([128, 2], F32, tag="mv")
                nc.vector.bn_aggr(out=mv, in_=stats)
                nc.scalar.activation(out=mv[:, 1:2], in_=mv[:, 1:2], func=Act.Sqrt,
                                     bias=eps_sb[:, 0:1], scale=1.0)
                nc.vector.reciprocal(out=mv[:, 1:2], in_=mv[:, 1:2])
                nmrs = workp.tile([128, 1], F32, tag="nmrs")
                nc.vector.tensor_tensor(out=nmrs, in0=mv[:, 0:1], in1=mv[:, 1:2],
                                        op=Alu.mult)
                nc.vector.tensor_scalar(out=nmrs, in0=nmrs, scalar1=-1.0,
                                        scalar2=None, op0=Alu.mult)
                # (x - m) * rs  ==  x * rs + (-m * rs), on ScalarE
                if ln_trivial:
                    nc.scalar.activation(out=out_ap, in_=src_ap, func=Act.Identity,
                                         bias=nmrs[:, 0:1], scale=mv[:, 1:2])
                else:
                    tmp = workp.tile([128, 128], F32, tag="lntmp")
                    nc.scalar.activation(out=tmp, in_=src_ap, func=Act.Identity,
                                         bias=nmrs[:, 0:1], scale=mv[:, 1:2])
                    nc.vector.tensor_tensor(out=tmp, in0=tmp,
                                            in1=ln_g_sb[:, il, :], op=Alu.mult)
                    nc.vector.tensor_tensor(out=out_ap, in0=tmp,
                                            in1=ln_b_sb[:, il, :], op=Alu.add)

            def store_x(b, last_layer):
                r0 = b * 128
                nv = 128 if b < NBLK - 1 else LAST_VALID
                if last_layer:
                    nc.sync.dma_start(out=y_out[r0:r0 + nv, :], in_=x_tiles[b][:nv, :])
                else:
                    nc.sync.dma_start(out=x_my_bf[r0:r0 + nv, :],
                                      in_=x_tiles[b][:nv, :].bitcast(mybir.dt.float32r))

            def emit_ag(part, parity):
                lo = SEG_R0[part]
                hi = lo + SEG_NR[part]
                nc.gpsimd.collective_compute(
                    "AllGather", Alu.bypass,
                    replica_groups=[list(range(NCORES))],
                    ins=[x_my_bf[lo:hi, :]], outs=[x_tbl[part][parity][:, :]])

            def maybe_ag(b, parity):
                for part in range(NSEG):
                    if b == SEG_B0[part] + SEG_NB[part] - 1:
                        emit_ag(part, parity)

            # ---- embedding stage ----
            embctx = tc.tile_pool(name="embp", bufs=1)
            embp = embctx.__enter__()
            pos_tiles = []
            done = 0
            while done < NBLK:
                n_t = min(CHUNK, NBLK - done)
                pg = embp.tile([128, CHUNK, 128], F32, name=f"posg{done}")
                nc.gpsimd.dma_gather(
                    pg[:, 0:n_t, :], pos_table_in[:, :],
                    pos_idx_sb[:, done * 8:done * 8 + n_t * 8],
                    n_t * 128, n_t * 128, 128, queue_num=next_q())
                pos_tiles.append(pg)
                done += n_t

            ne_r = node_emb_in[0:(NBLK - 1) * 128, :].rearrange("(t p) d -> p t d", p=128)
            for b in range(NBLK):
                bc, bw = b // CHUNK, b % CHUNK
                if bw == 0:
                    n_t = min(CHUNK, NBLK - b)
                    et = embp.tile([128, CHUNK, 128], F32, tag="embt", bufs=2,
                                    name=f"embt{b}")
                    if b + n_t == NBLK:
                        nc.vector.memset(et[:, n_t - 1, :], 0.0)
                        if n_t > 1:
                            nc.sync.dma_start(out=et[:, 0:n_t - 1, :],
                                              in_=ne_r[:, b:b + n_t - 1, :])
                        nc.sync.dma_start(out=et[:LAST_VALID, n_t - 1, :],
                                          in_=node_emb_in[(NBLK - 1) * 128:NLOC, :])
                    else:
                        nc.sync.dma_start(out=et[:, 0:n_t, :], in_=ne_r[:, b:b + n_t, :])
                    t2w = embp.tile([128, CHUNK, 128], F32, tag="embt2", bufs=2,
                                     name=f"embt2{b}")
                    nc.vector.tensor_scalar(
                        out=t2w[:, 0:n_t, :].rearrange("p t d -> p (t d)"),
                        in0=et[:, 0:n_t, :].rearrange("p t d -> p (t d)"),
                        scalar1=SQRT_D, scalar2=None, op0=Alu.mult)
                    nc.vector.tensor_tensor(
                        out=t2w[:, 0:n_t, :].rearrange("p t d -> p (t d)"),
                        in0=t2w[:, 0:n_t, :].rearrange("p t d -> p (t d)"),
                        in1=pos_tiles[bc][:, 0:n_t, :].rearrange("p t d -> p (t d)"),
                        op=Alu.add)
                    cur_t2w = t2w
                layer_norm(cur_t2w[:, bw, :], 0, x_tiles[b])
                store_x(b, last_layer=False)
                maybe_ag(b, 0)

            embctx.__exit__(None, None, None)
            _gctx = [tc.tile_pool(name=f"g{i}", bufs=GBUFS[i]) for i in range(3)]
            _hctx = tc.tile_pool(name="hpool", bufs=3)
            g0p, g1p, g2p = [c.__enter__() for c in _gctx]
            hp = _hctx.__enter__()
            gpools = [g0p, g1p, g2p]
            span_max_h = [max(1, int(spans[:, h].max())) for h in range(NSEG)]

            # ---- layers ----
            for il in range(L):
                par = il % 2
                srcs = [x_tbl[h][par][:, :] for h in range(NSEG)]
                g_chunks = [{} for _ in range(NSEG)]
                issued = [0] * NSEG
                n_chunks = [(T[h] + CHUNK - 1) // CHUNK for h in range(NSEG)]

                def issue_chunk(h, ci, il=il, srcs=srcs, g_chunks=g_chunks):
                    t0 = ci * CHUNK
                    n_t = min(CHUNK, T[h] - t0)
                    g = gpools[h].tile([128, CHUNK, 128], mybir.dt.float32r,
                                       tag=f"g{h}", name=f"g{h}_{il}_{ci}")
                    nc.gpsimd.dma_gather(
                        g[:, 0:n_t, :], srcs[h],
                        idx_sb[h][:, t0 * 8:(t0 + n_t) * 8],
                        n_t * 128, n_t * 128, 128, queue_num=next_q())
                    g_chunks[h][ci] = g

                for b in range(NBLK):
                    for h in range(NSEG):
                        if spans[b][h] > 0:
                            need = min((int(tile0[b][h] + spans[b][h]) + CHUNK - 1) // CHUNK,
                                       n_chunks[h])
                            while issued[h] < need:
                                issue_chunk(h, issued[h])
                                issued[h] += 1

                    # one-hot tiles for this block (all streams)
                    hts = []
                    for h in range(NSEG):
                        sp = int(spans[b][h])
                        if sp == 0:
                            hts.append(None)
                            continue
                        ht = hp.tile([128, span_max_h[h], 128], mybir.dt.float32r,
                                     tag=f"h{h}", name=f"h{h}_{il}_{b}")
                        co = int(col_offs[b][h])
                        csl = col_sb[h][:, co:co + sp]
                        cbc = bass.AP(tensor=csl.tensor, offset=csl.offset,
                                      ap=[csl.ap[0], [csl.ap[1][0], sp], [0, 128]])
                        nc.vector.tensor_tensor(
                            out=ht[:, 0:sp, :].rearrange("p t c -> p (t c)"),
                            in0=iota_w[:, 0:sp * 128], in1=cbc, op=Alu.is_equal)
                        hts.append(ht)

                    aggT = psump.tile([128, 128], F32, tag="aggT")
                    n_mm = int(spans[b].sum())
                    k = 0
                    for h in range(NSEG):
                        for j in range(int(spans[b][h])):
                            t = int(tile0[b][h]) + j
                            ci, w = t // CHUNK, t % CHUNK
                            nc.tensor.matmul(
                                aggT, g_chunks[h][ci][:, w, :], hts[h][:, j, :],
                                start=(k == 0), stop=(k == n_mm - 1))
                            k += 1
                    aggT_sb = workp.tile([128, 128], F32, tag="aggT_sb")
                    if n_mm == 0:
                        nc.vector.memset(aggT_sb, 0.0)
                    else:
                        nc.scalar.copy(out=aggT_sb, in_=aggT)

                    xT = psump.tile([128, 128], F32, tag="xT")
                    nc.tensor.transpose(xT, x_tiles[b], ident)
                    xT_sb = workp.tile([128, 128], F32, tag="xT_sb")
                    nc.vector.tensor_copy(out=xT_sb, in_=xT)

                    h1T = psump.tile([128, 128], F32, tag="h1T")
                    nc.tensor.matmul(h1T, wl_sb[:, il, :], aggT_sb, start=True, stop=False)
                    nc.tensor.matmul(h1T, wr_sb[:, il, :], xT_sb, start=False, stop=True)

                    hT_sb = workp.tile([128, 128], F32, tag="hT_sb")
                    nc.scalar.activation(out=hT_sb, in_=h1T, func=Act.Relu,
                                         bias=blt_sb[:, il:il + 1], scale=1.0)

                    hps = psump.tile([128, 128], F32, tag="hps")
                    nc.tensor.transpose(hps, hT_sb, ident)

                    r = workp.tile([128, 128], F32, tag="r")
                    nc.vector.tensor_tensor(out=r, in0=hps, in1=x_tiles[b], op=Alu.add)
                    layer_norm(r, il + 1, x_tiles[b])
                    store_x(b, last_layer=(il == L - 1))
                    if il < L - 1:
                        maybe_ag(b, (il + 1) % 2)

            _hctx.__exit__(None, None, None)
            for c in reversed(_gctx):
                c.__exit__(None, None, None)

    nc.compile()
    return nc


def kernel(node_emb, pos, edge, pos_table, Wl, bl, Wr,
           emb_ln_g, emb_ln_b, hid_ln_g, hid_ln_b):
    node_emb = np.asarray(node_emb, np.float32)
    pos = np.asarray(pos, np.int32)
    edge = np.asarray(edge, np.int64)
    pos_table = np.asarray(pos_table, np.float32)
    Wl = np.asarray(Wl, np.float32)
    bl = np.asarray(bl, np.float32)
    Wr = np.asarray(Wr, np.float32)
    ln_g = np.stack([np.asarray(emb_ln_g, np.float32)] +
                    [np.asarray(hid_ln_g[i], np.float32) for i in range(L)])
    ln_b = np.stack([np.asarray(emb_ln_b, np.float32)] +
                    [np.asarray(hid_ln_b[i], np.float32) for i in range(L)])

    new_row = _balance_perm(edge)
    edge_p = new_row[edge]
    prep = _prepare(edge_p)
    ln_trivial = bool(np.all(ln_g == 1.0) and np.all(ln_b == 0.0))
    nc = _build(prep, ln_trivial)
    inv = np.empty(N, np.int64)
    inv[new_row] = np.arange(N)

    blt = np.ascontiguousarray(bl.T)          # [D, L]
    in_maps = []
    for c in range(NCORES):
        pc = prep['per_core'][c]
        pos_c = pos[inv[c * NLOC:(c + 1) * NLOC]].astype(np.int16)
        pos_pad = np.zeros(6272, np.int16)
        pos_pad[:NLOC] = pos_c
        im = dict(
            node_emb_in=np.ascontiguousarray(node_emb[inv[c * NLOC:(c + 1) * NLOC]]),
            pos_idx_in=_wrap_idx(pos_pad),
            pos_table_in=pos_table,
            wl_in=Wl, wr_in=Wr, blt_in=blt,
            ln_g_in=ln_g, ln_b_in=ln_b,
        )
        for h in range(NSEG):
            im[f"idx{h}_in"] = pc['idx'][h]
            im[f"col{h}_in"] = np.ascontiguousarray(pc['col'][h])
        in_maps.append(im)

    trace = os.environ.get("GNN_TRACE") == "1"
    if trace:
        try:
            import trnprof
            trnprof.install()
        except Exception:
            trace = False
    res = run_bass_kernel_spmd(nc, in_maps, core_ids=list(range(NCORES)), trace=trace)
    kernel.last_result = res
    y = np.concatenate([res.results[c]["y_out"] for c in range(NCORES)], axis=0)
    return y[new_row]


# revision 23
# speedup vs baseline: 1.2489x; 1.2489x over previous
"""GNN message-passing (SAGEConv x3 + LayerNorm) Trainium2 kernel, 8 NeuronCores.

Strategy (graph/data parallel, per sharding hint):
  - Nodes sharded 8 ways by contiguous ranges (6250/core); edges owned by dst core.
  - Per layer: bf16 AllGather of node features, split into 3 row segments
    (blocks 0-23 / 24-46 / 47-48 of each core) emitted as soon as their blocks
    finish, so next-layer gathers for the bulk segments overlap the layer tail
    and the AllGather latency hides behind the tiny trailing segment ->
    per-core dma_gather of x[src] (edges sorted by (dst_block, segment, src))
    -> segment-sum via one-hot matmuls on the TensorEngine accumulating into
    PSUM per 128-dst block -> fused agg@Wl + x@Wr in transposed layout ->
    relu+bias on ScalarE -> residual+LN on DVE/ScalarE.
  - Gather indices are int16; each AllGather segment doubles as an index
    table (< 32768 rows each). Block edge lists are packed edge-granular
    with uniform-across-cores capacities; boundary tiles are shared between
    adjacent blocks via separate one-hot matrices. A host-side permutation
    shuffles nodes within each (core, segment) to equalize per-(block,segment)
    edge counts across cores (SPMD: one program for all 8 cores).
"""
import os
import sys

for _p in ('/opt/trn_rl_repo', '/root/.axon_site/_ro/trn_rl_repo'):
    if os.path.isdir(_p) and _p not in sys.path:
        sys.path.insert(0, _p)

import numpy as np

import concourse.bacc as bacc
import concourse.bass as bass
import concourse.tile as tile
from concourse import mybir
from concourse.bass_utils import run_bass_kernel_spmd
from concourse.masks import make_identity

N, E, D, L, POS_VOC = 50000, 800000, 128, 3, 1024
NCORES = 8
NLOC = N // NCORES            # 6250 nodes per core
NBLK = (NLOC + 127) // 128    # 49 dst blocks per core (last has 106)
LAST_VALID = NLOC - (NBLK - 1) * 128   # 106
SEG_B0 = [0, 24, 47]          # first block of each AllGather segment
SEG_NB = [24, 23, 2]          # blocks per segment
SEG_R0 = [0, 3072, 6016]      # first row of each segment within a core
SEG_NR = [3072, 2944, 234]    # rows per segment within a core
NSEG = 3
EPS = 1e-5
SQRT_D = float(np.sqrt(D))
CHUNK = 8                     # gather tiles per dma_gather call (1024 idxs max)
NQ = 4                        # SWDGE queues (parallel desc rings)
GBUFS = [int(x) for x in os.environ.get('GNN_GBUFS', '4,4,2').split(',')]

F32 = mybir.dt.float32
BF16 = mybir.dt.bfloat16
I16 = mybir.dt.int16
I32 = mybir.dt.int32
Alu = mybir.AluOpType
Act = mybir.ActivationFunctionType


def _wrap_idx(idx):
    """int16 gather index layout: logical j at [j%16, j//16], replicated to 128 partitions."""
    idx = np.asarray(idx, np.int16)
    n = len(idx)
    assert n % 16 == 0
    w = idx.reshape(-1, 16).T.copy()          # [16, n//16]
    return np.tile(w, (8, 1))                 # [128, n//16]


def _seg_of_rows(r):
    s = np.zeros_like(r)
    s[r >= SEG_R0[1]] = 1
    s[r >= SEG_R0[2]] = 2
    return s


def _balance_perm(edge):
    """Within each (core, segment), shuffle nodes among the segment's blocks so
    per-(core, block, src-segment) in-edge counts equalize across cores.
    Returns new_row[v] (global node id -> new global row)."""
    src = np.asarray(edge[0], np.int64)
    dst = np.asarray(edge[1], np.int64)
    s_core = src // NLOC
    s_r = src - s_core * NLOC
    cls = _seg_of_rows(s_r)
    deg = np.zeros((NSEG, N), np.int64)
    for h in range(NSEG):
        deg[h] = np.bincount(dst[cls == h], minlength=N)
    degt = deg.sum(axis=0)
    new_row = np.zeros(N, np.int64)
    for c in range(NCORES):
        base = c * NLOC
        for h in range(NSEG):
            nodes = np.arange(base + SEG_R0[h], base + SEG_R0[h] + SEG_NR[h])
            nodes = nodes[np.argsort(-degt[nodes], kind='stable')]
            nb = SEG_NB[h]
            capn = np.full(nb, 128, np.int64)
            if h == NSEG - 1:
                capn[nb - 1] = LAST_VALID
            load = np.zeros(nb, np.int64)
            cnt = np.zeros(nb, np.int64)
            slots = [[] for _ in range(nb)]
            for v in nodes:
                t = np.where(cnt < capn, load, 1 << 60)
                j = int(np.argmin(t))
                slots[j].append(v)
                load[j] += degt[v]
                cnt[j] += 1
            for j in range(nb):
                r0 = (SEG_B0[h] + j) * 128
                for k, v in enumerate(slots[j]):
                    new_row[v] = base + r0 + k
    return new_row


def _prepare(edge):
    """Host-side index preprocessing: per-core gather streams (one per segment
    table) with edge-granular per-block capacities uniform across cores, plus
    per-block one-hot column data (boundary tiles shared between blocks)."""
    src = np.asarray(edge[0], np.int64)
    dst = np.asarray(edge[1], np.int64)
    core = dst // NLOC
    dl = dst - core * NLOC
    blk = dl // 128
    col = dl - blk * 128
    s_core = src // NLOC
    s_r = src - s_core * NLOC
    cls = _seg_of_rows(s_r)
    seg_r0 = np.array(SEG_R0)[cls]
    seg_nr = np.array(SEG_NR)[cls]
    idxv = s_core * seg_nr + (s_r - seg_r0)

    # sort edges by (core, block, segment, src idx)
    key = (((core * NBLK + blk) * NSEG + cls) * (NCORES * max(SEG_NR) + 1)) + idxv
    order = np.argsort(key, kind='stable')
    g_idx, g_col = idxv[order], col[order]

    ngroups = NCORES * NBLK * NSEG
    gid = (core * NBLK + blk) * NSEG + cls
    counts = np.bincount(gid[order], minlength=ngroups).reshape(NCORES, NBLK, NSEG)
    caps = counts.max(axis=0)                              # [NBLK, NSEG]
    offs = np.zeros((NBLK + 1, NSEG), np.int64)
    offs[1:] = np.cumsum(caps, axis=0)
    T = [int((offs[NBLK][h] + 127) // 128) for h in range(NSEG)]

    spans = np.zeros((NBLK, NSEG), np.int64)
    col_offs = np.zeros((NBLK, NSEG), np.int64)
    tile0 = np.zeros((NBLK, NSEG), np.int64)
    acc = [0] * NSEG
    for b in range(NBLK):
        for h in range(NSEG):
            o0, o1 = int(offs[b][h]), int(offs[b + 1][h])
            t0 = o0 // 128
            t1 = (o1 - 1) // 128 + 1 if o1 > o0 else t0
            tile0[b][h] = t0
            spans[b][h] = t1 - t0
            col_offs[b][h] = acc[h]
            acc[h] += t1 - t0
    CS = [max(int(acc[h]), 1) for h in range(NSEG)]

    starts = np.concatenate([[0], np.cumsum(counts.reshape(-1))])
    per_core = []
    for c in range(NCORES):
        idx_s = [np.zeros(max(T[h], 1) * 128, np.int16) for h in range(NSEG)]
        col_s = [np.full(CS[h] * 128, -1.0, np.float32) for h in range(NSEG)]
        for b in range(NBLK):
            for h in range(NSEG):
                g = (c * NBLK + b) * NSEG + h
                s0, s1 = starts[g], starts[g + 1]
                n_e = s1 - s0
                if n_e == 0:
                    continue
                p0 = int(offs[b][h])
                idx_s[h][p0:p0 + n_e] = g_idx[s0:s1]
                q0 = int(col_offs[b][h]) * 128 + (p0 - int(tile0[b][h]) * 128)
                col_s[h][q0:q0 + n_e] = g_col[s0:s1]
        per_core.append(dict(
            idx=[_wrap_idx(idx_s[h]) for h in range(NSEG)],
            col=[col_s[h].reshape(-1, 128).T.copy() for h in range(NSEG)],
        ))
    return dict(T=T, CS=CS, spans=spans, tile0=tile0, col_offs=col_offs,
                per_core=per_core, span_max=max(1, int(spans.max())))


def _build(prep, ln_trivial):
    T, CS = prep['T'], prep['CS']
    spans, tile0, col_offs = prep['spans'], prep['tile0'], prep['col_offs']
    SPAN_MAX = prep['span_max']

    nc = bacc.Bacc('TRN2', num_devices=NCORES, num_swdge_queues=NQ,
                   dynamic_dma_scratch_size=int(os.environ.get('GNN_SCRATCH', '32768')))

    # ---- I/O ----
    node_emb_in = nc.dram_tensor("node_emb_in", [NLOC, D], F32, kind="ExternalInput")
    pos_idx_in = nc.dram_tensor("pos_idx_in", [128, 6272 // 16], I16, kind="ExternalInput")
    pos_table_in = nc.dram_tensor("pos_table_in", [POS_VOC, D], F32, kind="ExternalInput")
    idx_in = [nc.dram_tensor(f"idx{h}_in", [128, max(T[h], 1) * 8], I16,
                             kind="ExternalInput") for h in range(NSEG)]
    col_in = [nc.dram_tensor(f"col{h}_in", [128, CS[h]], F32, kind="ExternalInput")
              for h in range(NSEG)]
    wl_in = nc.dram_tensor("wl_in", [L, D, D], F32, kind="ExternalInput")
    wr_in = nc.dram_tensor("wr_in", [L, D, D], F32, kind="ExternalInput")
    blt_in = nc.dram_tensor("blt_in", [D, L], F32, kind="ExternalInput")
    ln_g_in = nc.dram_tensor("ln_g_in", [L + 1, D], F32, kind="ExternalInput")
    ln_b_in = nc.dram_tensor("ln_b_in", [L + 1, D], F32, kind="ExternalInput")
    y_out = nc.dram_tensor("y_out", [NLOC, D], F32, kind="ExternalOutput")

    F16 = mybir.dt.float16
    x_my_bf = nc.dram_tensor("x_my_bf", [NLOC, D], F16)
    # ping-pong gather tables by layer parity (avoids WAR with in-flight gathers)
    x_tbl = [[nc.dram_tensor(f"x_tbl{h}_{p}", [NCORES * SEG_NR[h], D],
                             F16, addr_space="Shared")
              for p in range(2)] for h in range(NSEG)]

    with tile.TileContext(nc) as tc:
        with tc.tile_pool(name="const", bufs=1) as constp, \
             tc.tile_pool(name="xres", bufs=1) as xresp, \
             tc.tile_pool(name="work", bufs=3) as workp, \
             tc.tile_pool(name="psum", bufs=2, space="PSUM") as psump:

            # ---- constants ----
            idx_sb, col_sb = [], []
            for h in range(NSEG):
                isb = constp.tile([128, max(T[h], 1) * 8], I16, name=f"idx_sb{h}")
                nc.sync.dma_start(out=isb, in_=idx_in[h][:, :])
                idx_sb.append(isb)
                csb = constp.tile([128, CS[h]], F32, name=f"col_sb{h}")
                nc.sync.dma_start(out=csb, in_=col_in[h][:, :])
                col_sb.append(csb)
            pos_idx_sb = constp.tile([128, 6272 // 16], I16)
            nc.sync.dma_start(out=pos_idx_sb, in_=pos_idx_in[:, :])

            wl_sb = constp.tile([128, L, D], F32)
            nc.sync.dma_start(out=wl_sb, in_=wl_in[:, :, :].rearrange("l c f -> c l f"))
            wr_sb = constp.tile([128, L, D], F32)
            nc.sync.dma_start(out=wr_sb, in_=wr_in[:, :, :].rearrange("l c f -> c l f"))
            blt_sb = constp.tile([128, L], F32)
            nc.sync.dma_start(out=blt_sb, in_=blt_in[:, :])

            def bcast128(dram_row):   # replicate a [D] DRAM row across 128 partitions
                return bass.AP(tensor=dram_row.tensor, offset=dram_row.offset,
                               ap=[[0, 128]] + dram_row.ap)

            ln_g_sb = constp.tile([128, L + 1, D], F32)
            ln_b_sb = constp.tile([128, L + 1, D], F32)
            for i in range(L + 1):
                nc.sync.dma_start(out=ln_g_sb[:, i, :], in_=bcast128(ln_g_in[i, :]))
                nc.sync.dma_start(out=ln_b_sb[:, i, :], in_=bcast128(ln_b_in[i, :]))

            eps_sb = constp.tile([128, 1], F32)
            nc.vector.memset(eps_sb, EPS)
            ident = constp.tile([128, 128], F32)
            make_identity(nc, ident)

            iota_i = constp.tile([128, SPAN_MAX * 128], I32)
            nc.gpsimd.iota(iota_i, pattern=[[0, SPAN_MAX], [1, 128]], base=0,
                           channel_multiplier=0)
            iota_w = constp.tile([128, SPAN_MAX * 128], F32)
            nc.vector.tensor_copy(out=iota_w, in_=iota_i)

            # persistent x tiles (f32), one per block for fine-grained deps
            x_tiles = [xresp.tile([128, 128], F32, tag=f"x{t}", name=f"xres{t}")
                       for t in range(NBLK)]

            qn = [0]
            def next_q():
                q = qn[0] % NQ
                qn[0] += 1
                return q

            def layer_norm(src_ap, il, out_ap):
                stats = workp.tile([128, 6], F32, tag="stats")
                nc.vector.bn_stats(out=stats, in_=src_ap)
                mv = workp.tile([128, 2], F32, tag="mv")
                nc.vector.bn_aggr(out=mv, in_=stats)
                nc.scalar.activation(out=mv[:, 1:2], in_=mv[:, 1:2], func=Act.Sqrt,
                                     bias=eps_sb[:, 0:1], scale=1.0)
                nc.vector.reciprocal(out=mv[:, 1:2], in_=mv[:, 1:2])
                nmrs = workp.tile([128, 1], F32, tag="nmrs")
                nc.vector.tensor_tensor(out=nmrs, in0=mv[:, 0:1], in1=mv[:, 1:2],
                                        op=Alu.mult)
                nc.vector.tensor_scalar(out=nmrs, in0=nmrs, scalar1=-1.0,
                                        scalar2=None, op0=Alu.mult)
                # (x - m) * rs  ==  x * rs + (-m * rs), on ScalarE
                if ln_trivial:
                    nc.scalar.activation(out=out_ap, in_=src_ap, func=Act.Identity,
                                         bias=nmrs[:, 0:1], scale=mv[:, 1:2])
                else:
                    tmp = workp.tile([128, 128], F32, tag="lntmp")
                    nc.scalar.activation(out=tmp, in_=src_ap, func=Act.Identity,
                                         bias=nmrs[:, 0:1], scale=mv[:, 1:2])
                    nc.vector.tensor_tensor(out=tmp, in0=tmp,
                                            in1=ln_g_sb[:, il, :], op=Alu.mult)
                    nc.vector.tensor_tensor(out=out_ap, in0=tmp,
                                            in1=ln_b_sb[:, il, :], op=Alu.add)

            def store_x(b, last_layer):
                r0 = b * 128
                nv = 128 if b < NBLK - 1 else LAST_VALID
                if last_layer:
                    nc.sync.dma_start(out=y_out[r0:r0 + nv, :], in_=x_tiles[b][:nv, :])
                else:
                    xf16 = workp.tile([128, 128], mybir.dt.float16, tag="xf16")
                    nc.scalar.copy(out=xf16, in_=x_tiles[b])
                    nc.sync.dma_start(out=x_my_bf[r0:r0 + nv, :], in_=xf16[:nv, :])

            def emit_ag(part, parity):
                lo = SEG_R0[part]
                hi = lo + SEG_NR[part]
                nc.gpsimd.collective_compute(
                    "AllGather", Alu.bypass,
                    replica_groups=[list(range(NCORES))],
                    ins=[x_my_bf[lo:hi, :]], outs=[x_tbl[part][parity][:, :]])

            def maybe_ag(b, parity):
                for part in range(NSEG):
                    if b == SEG_B0[part] + SEG_NB[part] - 1:
                        emit_ag(part, parity)

            # ---- embedding stage ----
            embctx = tc.tile_pool(name="embp", bufs=1)
            embp = embctx.__enter__()
            pos_tiles = []
            done = 0
            while done < NBLK:
                n_t = min(CHUNK, NBLK - done)
                pg = embp.tile([128, CHUNK, 128], F32, name=f"posg{done}")
                nc.gpsimd.dma_gather(
                    pg[:, 0:n_t, :], pos_table_in[:, :],
                    pos_idx_sb[:, done * 8:done * 8 + n_t * 8],
                    n_t * 128, n_t * 128, 128, queue_num=next_q())
                pos_tiles.append(pg)
                done += n_t

            ne_r = node_emb_in[0:(NBLK - 1) * 128, :].rearrange("(t p) d -> p t d", p=128)
            for b in range(NBLK):
                bc, bw = b // CHUNK, b % CHUNK
                if bw == 0:
                    n_t = min(CHUNK, NBLK - b)
                    et = embp.tile([128, CHUNK, 128], F32, tag="embt", bufs=2,
                                    name=f"embt{b}")
                    if b + n_t == NBLK:
                        nc.vector.memset(et[:, n_t - 1, :], 0.0)
                        if n_t > 1:
                            nc.sync.dma_start(out=et[:, 0:n_t - 1, :],
                                              in_=ne_r[:, b:b + n_t - 1, :])
                        nc.sync.dma_start(out=et[:LAST_VALID, n_t - 1, :],
                                          in_=node_emb_in[(NBLK - 1) * 128:NLOC, :])
                    else:
                        nc.sync.dma_start(out=et[:, 0:n_t, :], in_=ne_r[:, b:b + n_t, :])
                    t2w = embp.tile([128, CHUNK, 128], F32, tag="embt2", bufs=2,
                                     name=f"embt2{b}")
                    nc.vector.tensor_scalar(
                        out=t2w[:, 0:n_t, :].rearrange("p t d -> p (t d)"),
                        in0=et[:, 0:n_t, :].rearrange("p t d -> p (t d)"),
                        scalar1=SQRT_D, scalar2=None, op0=Alu.mult)
                    nc.vector.tensor_tensor(
                        out=t2w[:, 0:n_t, :].rearrange("p t d -> p (t d)"),
                        in0=t2w[:, 0:n_t, :].rearrange("p t d -> p (t d)"),
                        in1=pos_tiles[bc][:, 0:n_t, :].rearrange("p t d -> p (t d)"),
                        op=Alu.add)
                    cur_t2w = t2w
                layer_norm(cur_t2w[:, bw, :], 0, x_tiles[b])
                store_x(b, last_layer=False)
                maybe_ag(b, 0)

            embctx.__exit__(None, None, None)
            _gctx = [tc.tile_pool(name=f"g{i}", bufs=GBUFS[i]) for i in range(3)]
            _hctx = tc.tile_pool(name="hpool", bufs=3)
            g0p, g1p, g2p = [c.__enter__() for c in _gctx]
            hp = _hctx.__enter__()
            gpools = [g0p, g1p, g2p]
            span_max_h = [max(1, int(spans[:, h].max())) for h in range(NSEG)]

            # ---- layers ----
            for il in range(L):
                par = il % 2
                srcs = [x_tbl[h][par][:, :] for h in range(NSEG)]
                g_chunks = [{} for _ in range(NSEG)]
                issued = [0] * NSEG
                n_chunks = [(T[h] + CHUNK - 1) // CHUNK for h in range(NSEG)]

                def issue_chunk(h, ci, il=il, srcs=srcs, g_chunks=g_chunks):
                    t0 = ci * CHUNK
                    n_t = min(CHUNK, T[h] - t0)
                    g = gpools[h].tile([128, CHUNK, 128], mybir.dt.float16,
                                       tag=f"g{h}", name=f"g{h}_{il}_{ci}")
                    nc.gpsimd.dma_gather(
                        g[:, 0:n_t, :], srcs[h],
                        idx_sb[h][:, t0 * 8:(t0 + n_t) * 8],
                        n_t * 128, n_t * 128, 128, queue_num=next_q())
                    g_chunks[h][ci] = g

                for b in range(NBLK):
                    for h in range(NSEG):
                        if spans[b][h] > 0:
                            need = min((int(tile0[b][h] + spans[b][h]) + CHUNK - 1) // CHUNK,
                                       n_chunks[h])
                            while issued[h] < need:
                                issue_chunk(h, issued[h])
                                issued[h] += 1

                    # one-hot tiles for this block (all streams)
                    hts = []
                    for h in range(NSEG):
                        sp = int(spans[b][h])
                        if sp == 0:
                            hts.append(None)
                            continue
                        ht = hp.tile([128, span_max_h[h], 128], mybir.dt.float16,
                                     tag=f"h{h}", name=f"h{h}_{il}_{b}")
                        co = int(col_offs[b][h])
                        csl = col_sb[h][:, co:co + sp]
                        cbc = bass.AP(tensor=csl.tensor, offset=csl.offset,
                                      ap=[csl.ap[0], [csl.ap[1][0], sp], [0, 128]])
                        nc.vector.tensor_tensor(
                            out=ht[:, 0:sp, :].rearrange("p t c -> p (t c)"),
                            in0=iota_w[:, 0:sp * 128], in1=cbc, op=Alu.is_equal)
                        hts.append(ht)

                    aggT = psump.tile([128, 128], F32, tag="aggT")
                    n_mm = int(spans[b].sum())
                    k = 0
                    for h in range(NSEG):
                        for j in range(int(spans[b][h])):
                            t = int(tile0[b][h]) + j
                            ci, w = t // CHUNK, t % CHUNK
                            nc.tensor.matmul(
                                aggT, g_chunks[h][ci][:, w, :], hts[h][:, j, :],
                                start=(k == 0), stop=(k == n_mm - 1))
                            k += 1
                    aggT_sb = workp.tile([128, 128], F32, tag="aggT_sb")
                    if n_mm == 0:
                        nc.vector.memset(aggT_sb, 0.0)
                    else:
                        nc.scalar.copy(out=aggT_sb, in_=aggT)

                    xT = psump.tile([128, 128], F32, tag="xT")
                    nc.tensor.transpose(xT, x_tiles[b], ident)
                    xT_sb = workp.tile([128, 128], F32, tag="xT_sb")
                    nc.vector.tensor_copy(out=xT_sb, in_=xT)

                    h1T = psump.tile([128, 128], F32, tag="h1T")
                    nc.tensor.matmul(h1T, wl_sb[:, il, :], aggT_sb, start=True, stop=False)
                    nc.tensor.matmul(h1T, wr_sb[:, il, :], xT_sb, start=False, stop=True)

                    hT_sb = workp.tile([128, 128], F32, tag="hT_sb")
                    nc.scalar.activation(out=hT_sb, in_=h1T, func=Act.Relu,
                                         bias=blt_sb[:, il:il + 1], scale=1.0)

                    hps = psump.tile([128, 128], F32, tag="hps")
                    nc.tensor.transpose(hps, hT_sb, ident)

                    r = workp.tile([128, 128], F32, tag="r")
                    nc.vector.tensor_tensor(out=r, in0=hps, in1=x_tiles[b], op=Alu.add)
                    layer_norm(r, il + 1, x_tiles[b])
                    store_x(b, last_layer=(il == L - 1))
                    if il < L - 1:
                        maybe_ag(b, (il + 1) % 2)

            _hctx.__exit__(None, None, None)
            for c in reversed(_gctx):
                c.__exit__(None, None, None)

    nc.compile()
    return nc


def kernel(node_emb, pos, edge, pos_table, Wl, bl, Wr,
           emb_ln_g, emb_ln_b, hid_ln_g, hid_ln_b):
    node_emb = np.asarray(node_emb, np.float32)
    pos = np.asarray(pos, np.int32)
    edge = np.asarray(edge, np.int64)
    pos_table = np.asarray(pos_table, np.float32)
    Wl = np.asarray(Wl, np.float32)
    bl = np.asarray(bl, np.float32)
    Wr = np.asarray(Wr, np.float32)
    ln_g = np.stack([np.asarray(emb_ln_g, np.float32)] +
                    [np.asarray(hid_ln_g[i], np.float32) for i in range(L)])
    ln_b = np.stack([np.asarray(emb_ln_b, np.float32)] +
                    [np.asarray(hid_ln_b[i], np.float32) for i in range(L)])

    new_row = _balance_perm(edge)
    edge_p = new_row[edge]
    prep = _prepare(edge_p)
    ln_trivial = bool(np.all(ln_g == 1.0) and np.all(ln_b == 0.0))
    nc = _build(prep, ln_trivial)
    inv = np.empty(N, np.int64)
    inv[new_row] = np.arange(N)

    blt = np.ascontiguousarray(bl.T)          # [D, L]
    in_maps = []
    for c in range(NCORES):
        pc = prep['per_core'][c]
        pos_c = pos[inv[c * NLOC:(c + 1) * NLOC]].astype(np.int16)
        pos_pad = np.zeros(6272, np.int16)
        pos_pad[:NLOC] = pos_c
        im = dict(
            node_emb_in=np.ascontiguousarray(node_emb[inv[c * NLOC:(c + 1) * NLOC]]),
            pos_idx_in=_wrap_idx(pos_pad),
            pos_table_in=pos_table,
            wl_in=Wl, wr_in=Wr, blt_in=blt,
            ln_g_in=ln_g, ln_b_in=ln_b,
        )
        for h in range(NSEG):
            im[f"idx{h}_in"] = pc['idx'][h]
            im[f"col{h}_in"] = np.ascontiguousarray(pc['col'][h])
        in_maps.append(im)

    trace = os.environ.get("GNN_TRACE") == "1"
    if trace:
        try:
            import trnprof
            trnprof.install()
        except Exception:
            trace = False
    res = run_bass_kernel_spmd(nc, in_maps, core_ids=list(range(NCORES)), trace=trace)
    kernel.last_result = res
    y = np.concatenate([res.results[c]["y_out"] for c in range(NCORES)], axis=0)
    return y[new_row]
